# revision 20
# baseline (speedup 1.0000x reference)
"""SE(3) compose-scan Trainium2 kernel (nn_ComposeRt).

x [131072, 32, 3, 4] fp32 -> cumulative compose along axis 1:
out[b,0] = x[b,0]; out[b,n] = out[b,n-1] o x[b,n],
[rA|tA] o [rB|tB] = [rA@rB | tA + rA@tB].

Sharding: pure data parallel over batch across 8 NeuronCores.
Per core: batch b_local = t*(P*F) + p*F + f (mega-tile t, partition p,
slot f). DRAM I/O blocks [MEGA*HALVES, P, F*NSUB*12]; block (t, h) holds
n-range [h*NSUB, (h+1)*NSUB), SBUF layout [p][f][n][i*4+j].

Variants:
- "dve": per scan step, six vector-engine tensor ops (3 broadcast
  multiplies, 2 accumulate adds, translation add) batched over (f, i, j).
- "cumsum": the scalar engine materializes both operands of all nine
  rotation products as contiguous per-partition streams (A replicated
  over j, B replicated over i); one custom DVE op computes the running
  sum of products over the stream; a strided subtract of group
  boundaries extracts the nine dot products; a small add applies the
  carried translation. 51 instead of 63 DVE element-cycles per compose
  and 3 instead of 6 DVE instructions per step.
"""

import sys

if "/opt/trn_rl_repo" not in sys.path:
    sys.path.insert(0, "/opt/trn_rl_repo")

import numpy as np

import concourse.bacc as bacc
import concourse.mybir as mybir
from concourse import bass_utils, dve_ops
from concourse.dve_ops import DveOp
from concourse.dve_spec import AluOp, Spec, Src0, Src1, lower, scan
from concourse.dve_uop import DveOpSpec
from concourse.tile import TileContext

P = 128
N = 32
N_CORES = 8
B = 131072

# tunables
VARIANT = "soav"  # "dve" | "cumsum" | "colsplit" | "split2" | "soa" | "soav"
F = 128  # batch slots per partition per mega-tile
NSUB = 4  # n per sub-tile (DMA block)
MEGA = 1  # mega-tiles per core; MEGA*P*F == B // N_CORES
HALVES = N // NSUB
B_CORE = B // N_CORES
assert MEGA * P * F == B_CORE


def _register_cumsum_mul():
    """Runtime-register the custom DVE op out[k] = sum_{u<=k} in0[u]*in1[u]."""
    if any(op.name == "CUMSUM_MUL" for op in dve_ops.OPS):
        return next(op for op in dve_ops.OPS if op.name == "CUMSUM_MUL")

    def _ref(in0, in1, s0, s1, imm2):
        prod = in0.astype(np.float32) * in1.astype(np.float32)
        flat = prod.reshape(prod.shape[0], -1)
        return np.cumsum(flat, axis=-1).reshape(prod.shape)

    spec = Spec(body=scan(AluOp.ADD, Src0 * Src1), reference=_ref)
    shas = {}
    for ver in ("v3", "v4"):
        tmp = DveOpSpec(name="CUMSUM_MUL", opcode=0, uops=lower(spec, ver=ver), rd1_en=True)
        shas[ver] = tmp.sha(ver)
    op = DveOp("CUMSUM_MUL", spec, subdim=False, uops_sha=shas)
    dve_ops.OPS.append(op)
    dve_ops.CUSTOM_DVE_SPECS[op.name] = op.spec
    dve_ops._SUB_OPCODE_FOR_NAME[op.name] = (
        dve_ops._CUSTOM_DVE_ROW_BASE + len(dve_ops.OPS) - 1
    )
    return op


CUMSUM_MUL = None  # registered lazily by build() for the cumsum variant


class Cfg:
    def __init__(self, F=F, NSUB=NSUB, MEGA=MEGA, variant=VARIANT):
        self.F = F
        self.NSUB = NSUB
        self.MEGA = MEGA
        self.HALVES = N // NSUB
        self.B_CORE = MEGA * P * F
        self.variant = variant


def _step_dve(nc, ppool, C, A, Bm, sh):
    eng = nc.vector
    F_ = sh[1]
    tmp = ppool.tile([P, F_ * 12], mybir.dt.float32, tag="tk")
    tv = tmp.rearrange("p (f i j) -> p f i j", f=F_, i=3)
    eng.tensor_mul(
        out=C,
        in0=A[:, :, :, 0:1].broadcast_to(sh),
        in1=Bm[:, :, 0:1, :].broadcast_to(sh),
    )
    eng.tensor_mul(
        out=tv,
        in0=A[:, :, :, 1:2].broadcast_to(sh),
        in1=Bm[:, :, 1:2, :].broadcast_to(sh),
    )
    eng.tensor_add(out=C, in0=C, in1=tv)
    eng.tensor_mul(
        out=tv,
        in0=A[:, :, :, 2:3].broadcast_to(sh),
        in1=Bm[:, :, 2:3, :].broadcast_to(sh),
    )
    eng.tensor_add(out=C, in0=C, in1=tv)
    eng.tensor_add(out=C[:, :, :, 3], in0=C[:, :, :, 3], in1=A[:, :, :, 3])


def _step_colsplit(nc, ppool, C, A, Bm, sh):
    """Split the compose across engines: vector computes C's columns 0-2
    (rotation @ rotation), gpsimd computes column 3 (rotation @ tB + tA).
    Vector's step-n ops read only A's columns 0-2 (its own prior output),
    so the serial scan advances at vector speed; gpsimd trails one
    semaphore behind and never blocks it."""
    F_ = sh[1]
    sh3 = [P, F_, 3, 3]
    eng = nc.vector
    C3 = C[:, :, :, 0:3]
    tmp = ppool.tile([P, F_ * 9], mybir.dt.float32, tag="tk")
    tv = tmp.rearrange("p (f i j) -> p f i j", f=F_, i=3)
    eng.tensor_mul(
        out=C3,
        in0=A[:, :, :, 0:1].broadcast_to(sh3),
        in1=Bm[:, :, 0:1, 0:3].broadcast_to(sh3),
    )
    eng.tensor_mul(
        out=tv,
        in0=A[:, :, :, 1:2].broadcast_to(sh3),
        in1=Bm[:, :, 1:2, 0:3].broadcast_to(sh3),
    )
    eng.tensor_add(out=C3, in0=C3, in1=tv)
    eng.tensor_mul(
        out=tv,
        in0=A[:, :, :, 2:3].broadcast_to(sh3),
        in1=Bm[:, :, 2:3, 0:3].broadcast_to(sh3),
    )
    eng.tensor_add(out=C3, in0=C3, in1=tv)

    g = nc.gpsimd
    shv = [P, F_, 3]
    Ct = C[:, :, :, 3]
    tg = ppool.tile([P, F_ * 3], mybir.dt.float32, tag="tg")
    tgv = tg.rearrange("p (f i) -> p f i", f=F_)
    g.tensor_mul(
        out=Ct,
        in0=A[:, :, :, 0],
        in1=Bm[:, :, 0:1, 3].broadcast_to(shv),
    )
    g.tensor_mul(
        out=tgv,
        in0=A[:, :, :, 1],
        in1=Bm[:, :, 1:2, 3].broadcast_to(shv),
    )
    g.tensor_add(out=Ct, in0=Ct, in1=tgv)
    g.tensor_mul(
        out=tgv,
        in0=A[:, :, :, 2],
        in1=Bm[:, :, 2:3, 3].broadcast_to(shv),
    )
    g.tensor_add(out=Ct, in0=Ct, in1=tgv)
    g.tensor_add(out=Ct, in0=Ct, in1=A[:, :, :, 3])


def _step_cumsum(nc, epool, sbuf_S, C, A, Bm, sh):
    """A/Bm/C: [P, F, 3, 4] views; sbuf_S: persistent [P, 36F+3] scan buffer
    with S[:,0] pre-zeroed."""
    F_ = sh[1]
    G = 36 * F_
    aexp = epool.tile([P, G], mybir.dt.float32, tag="aexp")
    bexp = epool.tile([P, G], mybir.dt.float32, tag="bexp")
    # stream position = f*36 + i*12 + j*3 + k
    for k in range(3):
        a_out = aexp.rearrange("p (f i j k2) -> p f i j k2", f=F_, i=3, j=4)[
            :, :, :, :, k
        ]
        b_out = bexp.rearrange("p (f i j k2) -> p f i j k2", f=F_, i=3, j=4)[
            :, :, :, :, k
        ]
        nc.scalar.copy(out=a_out, in_=A[:, :, :, k : k + 1].broadcast_to(sh))
        nc.scalar.copy(out=b_out, in_=Bm[:, :, k : k + 1, :].broadcast_to(sh))
    s_out = sbuf_S[:, 1 : 1 + G]
    nc.vector._custom_dve(CUMSUM_MUL, out=s_out, in0=aexp[:], in1=bexp[:])
    minu = sbuf_S[:, 3 : 3 + G].rearrange("p (f g k) -> p f g k", f=F_, g=12)[
        :, :, :, 0
    ]
    subt = sbuf_S[:, 0:G].rearrange("p (f g k) -> p f g k", f=F_, g=12)[:, :, :, 0]
    cflat = C.rearrange("p f i j -> p f (i j)")
    nc.vector.tensor_tensor(
        out=cflat, in0=minu, in1=subt, op=mybir.AluOpType.subtract
    )
    nc.vector.tensor_add(out=C[:, :, :, 3], in0=C[:, :, :, 3], in1=A[:, :, :, 3])


def build_split2(cfg: Cfg):
    """Rotation (9 comps) and translation (3 comps) in separate tiles and
    separate DRAM outputs, so vector's rotation chain and gpsimd's
    translation chain never share a written tile: vector free-runs; gpsimd
    reads vector's rotations one step behind (one-way semaphore)."""
    F, NSUB, MEGA, HALVES = cfg.F, cfg.NSUB, cfg.MEGA, cfg.HALVES
    BLKX = F * NSUB * 12
    BLKR = F * NSUB * 9
    BLKT = F * NSUB * 3
    nc = bacc.Bacc("TRN2", target_bir_lowering=False, debug=False)
    x = nc.dram_tensor(
        "x", [MEGA * HALVES, P, BLKX], mybir.dt.float32, kind="ExternalInput"
    )
    yr = nc.dram_tensor(
        "yr", [MEGA * HALVES, P, BLKR], mybir.dt.float32, kind="ExternalOutput"
    )
    yt = nc.dram_tensor(
        "yt", [MEGA * HALVES, P, BLKT], mybir.dt.float32, kind="ExternalOutput"
    )

    with TileContext(nc) as tc:
        with (
            tc.tile_pool(name="xin", bufs=3) as xpool,
            tc.tile_pool(name="rotp", bufs=3) as rpool,
            tc.tile_pool(name="trap", bufs=3) as tpool,
            tc.tile_pool(name="work", bufs=3) as wpool,
        ):
            sh3 = [P, F, 3, 3]
            shv = [P, F, 3]
            for t in range(MEGA):
                prevR = prevT = None
                for h in range(HALVES):
                    xt = xpool.tile([P, BLKX], mybir.dt.float32, tag="x")
                    nc.sync.dma_start(out=xt[:], in_=x.ap()[t * HALVES + h])
                    rt = rpool.tile([P, BLKR], mybir.dt.float32, tag="r")
                    tt = tpool.tile([P, BLKT], mybir.dt.float32, tag="t")
                    xv = xt.rearrange("p (f n i j) -> p f n i j", f=F, n=NSUB, i=3)
                    rv = rt.rearrange("p (f n i j) -> p f n i j", f=F, n=NSUB, i=3)
                    tv = tt.rearrange("p (f n i) -> p f n i", f=F, n=NSUB)
                    for nl in range(NSUB):
                        if h == 0 and nl == 0:
                            nc.scalar.copy(out=rv[:, :, 0], in_=xv[:, :, 0, :, 0:3])
                            nc.scalar.copy(out=tv[:, :, 0], in_=xv[:, :, 0, :, 3])
                            continue
                        Ar = rv[:, :, nl - 1] if nl > 0 else prevR[:, :, NSUB - 1]
                        At = tv[:, :, nl - 1] if nl > 0 else prevT[:, :, NSUB - 1]
                        Bm = xv[:, :, nl]
                        Cr = rv[:, :, nl]
                        Ct = tv[:, :, nl]
                        eng = nc.vector
                        tmp = wpool.tile([P, F * 9], mybir.dt.float32, tag="tk")
                        tw = tmp.rearrange("p (f i j) -> p f i j", f=F, i=3)
                        eng.tensor_mul(
                            out=Cr,
                            in0=Ar[:, :, :, 0:1].broadcast_to(sh3),
                            in1=Bm[:, :, 0:1, 0:3].broadcast_to(sh3),
                        )
                        eng.tensor_mul(
                            out=tw,
                            in0=Ar[:, :, :, 1:2].broadcast_to(sh3),
                            in1=Bm[:, :, 1:2, 0:3].broadcast_to(sh3),
                        )
                        eng.tensor_add(out=Cr, in0=Cr, in1=tw)
                        eng.tensor_mul(
                            out=tw,
                            in0=Ar[:, :, :, 2:3].broadcast_to(sh3),
                            in1=Bm[:, :, 2:3, 0:3].broadcast_to(sh3),
                        )
                        eng.tensor_add(out=Cr, in0=Cr, in1=tw)

                        g = nc.gpsimd
                        tg = wpool.tile([P, F * 3], mybir.dt.float32, tag="tg")
                        tgv = tg.rearrange("p (f i) -> p f i", f=F)
                        g.tensor_mul(
                            out=Ct,
                            in0=Ar[:, :, :, 0],
                            in1=Bm[:, :, 0:1, 3].broadcast_to(shv),
                        )
                        g.tensor_mul(
                            out=tgv,
                            in0=Ar[:, :, :, 1],
                            in1=Bm[:, :, 1:2, 3].broadcast_to(shv),
                        )
                        g.tensor_add(out=Ct, in0=Ct, in1=tgv)
                        g.tensor_mul(
                            out=tgv,
                            in0=Ar[:, :, :, 2],
                            in1=Bm[:, :, 2:3, 3].broadcast_to(shv),
                        )
                        g.tensor_add(out=Ct, in0=Ct, in1=tgv)
                        g.tensor_add(out=Ct, in0=Ct, in1=At)
                    nc.sync.dma_start(out=yr.ap()[t * HALVES + h], in_=rt[:])
                    nc.sync.dma_start(out=yt.ap()[t * HALVES + h], in_=tt[:])
                    prevR, prevT = rv, tv
    nc.compile()
    return nc


def build_soa(cfg: Cfg):
    """Structure-of-arrays: every transform component is a contiguous
    [P, F] plane (f innermost, 512B runs), so all vector/gpsimd ops have
    stride-1 innermost access. Rotation (9 planes/n) on vector and
    translation (3 planes/n) on gpsimd live in separate tiles/outputs;
    gpsimd trails the vector rotation chain by one step."""
    F, NSUB, MEGA, HALVES = cfg.F, cfg.NSUB, cfg.MEGA, cfg.HALVES
    BLKX = NSUB * 12 * F
    BLKR = NSUB * 9 * F
    BLKT = NSUB * 3 * F
    nc = bacc.Bacc("TRN2", target_bir_lowering=False, debug=False)
    x = nc.dram_tensor(
        "x", [MEGA * HALVES, P, BLKX], mybir.dt.float32, kind="ExternalInput"
    )
    yr = nc.dram_tensor(
        "yr", [MEGA * HALVES, P, BLKR], mybir.dt.float32, kind="ExternalOutput"
    )
    yt = nc.dram_tensor(
        "yt", [MEGA * HALVES, P, BLKT], mybir.dt.float32, kind="ExternalOutput"
    )

    with TileContext(nc) as tc:
        with (
            tc.tile_pool(name="xin", bufs=3) as xpool,
            tc.tile_pool(name="rotp", bufs=3) as rpool,
            tc.tile_pool(name="trap", bufs=3) as tpool,
            tc.tile_pool(name="work", bufs=3) as wpool,
        ):
            sh33 = [P, 3, 3, F]
            sh3 = [P, 3, F]
            for t in range(MEGA):
                prevR = prevT = None
                for h in range(HALVES):
                    xt = xpool.tile([P, BLKX], mybir.dt.float32, tag="x")
                    nc.sync.dma_start(out=xt[:], in_=x.ap()[t * HALVES + h])
                    rt = rpool.tile([P, BLKR], mybir.dt.float32, tag="r")
                    tt = tpool.tile([P, BLKT], mybir.dt.float32, tag="t")
                    xq = xt.rearrange("p (n k j f) -> p n k j f", n=NSUB, k=3, j=4)
                    rq = rt.rearrange("p (n i j f) -> p n i j f", n=NSUB, i=3, j=3)
                    tq = tt.rearrange("p (n i f) -> p n i f", n=NSUB, i=3)
                    for nl in range(NSUB):
                        if h == 0 and nl == 0:
                            nc.scalar.copy(out=rq[:, 0], in_=xq[:, 0, :, 0:3])
                            nc.scalar.copy(out=tq[:, 0], in_=xq[:, 0, :, 3])
                            continue
                        Ar = rq[:, nl - 1] if nl > 0 else prevR[:, NSUB - 1]
                        At = tq[:, nl - 1] if nl > 0 else prevT[:, NSUB - 1]
                        Cr = rq[:, nl]
                        Ct = tq[:, nl]
                        eng = nc.vector
                        tmp = wpool.tile([P, 9 * F], mybir.dt.float32, tag="tk")
                        tw = tmp.rearrange("p (i j f) -> p i j f", i=3, j=3)
                        eng.tensor_mul(
                            out=Cr,
                            in0=Ar[:, :, 0:1, :].broadcast_to(sh33),
                            in1=xq[:, nl, 0:1, 0:3, :].broadcast_to(sh33),
                        )
                        eng.tensor_mul(
                            out=tw,
                            in0=Ar[:, :, 1:2, :].broadcast_to(sh33),
                            in1=xq[:, nl, 1:2, 0:3, :].broadcast_to(sh33),
                        )
                        eng.tensor_add(out=Cr, in0=Cr, in1=tw)
                        eng.tensor_mul(
                            out=tw,
                            in0=Ar[:, :, 2:3, :].broadcast_to(sh33),
                            in1=xq[:, nl, 2:3, 0:3, :].broadcast_to(sh33),
                        )
                        eng.tensor_add(out=Cr, in0=Cr, in1=tw)

                        g = nc.gpsimd
                        tg = wpool.tile([P, 3 * F], mybir.dt.float32, tag="tg")
                        tgv = tg.rearrange("p (i f) -> p i f", i=3)
                        g.tensor_mul(
                            out=Ct,
                            in0=Ar[:, :, 0, :],
                            in1=xq[:, nl, 0:1, 3, :].broadcast_to(sh3),
                        )
                        g.tensor_mul(
                            out=tgv,
                            in0=Ar[:, :, 1, :],
                            in1=xq[:, nl, 1:2, 3, :].broadcast_to(sh3),
                        )
                        g.tensor_add(out=Ct, in0=Ct, in1=tgv)
                        g.tensor_mul(
                            out=tgv,
                            in0=Ar[:, :, 2, :],
                            in1=xq[:, nl, 2:3, 3, :].broadcast_to(sh3),
                        )
                        g.tensor_add(out=Ct, in0=Ct, in1=tgv)
                        g.tensor_add(out=Ct, in0=Ct, in1=At)
                    nc.sync.dma_start(out=yr.ap()[t * HALVES + h], in_=rt[:])
                    nc.sync.dma_start(out=yt.ap()[t * HALVES + h], in_=tt[:])
                    prevR, prevT = rq, tq
    nc.compile()
    return nc


def build_soav(cfg: Cfg):
    """All-vector SoA: 12 component planes (i-row, j-col), each [P, F]
    contiguous. Per step 5 full-width ops (3 mul + 2 add over [P,3,4,F])
    compute A_rot @ [B_rot | tB]; one contiguous-plane add applies +tA.
    gpsimd is deliberately unused: its Q7 SBUF traffic degrades DVE
    throughput by ~50% when active."""
    F, NSUB, MEGA, HALVES = cfg.F, cfg.NSUB, cfg.MEGA, cfg.HALVES
    BLK = NSUB * 12 * F
    nc = bacc.Bacc("TRN2", target_bir_lowering=False, debug=False)
    x = nc.dram_tensor(
        "x", [MEGA * HALVES, P, BLK], mybir.dt.float32, kind="ExternalInput"
    )
    y = nc.dram_tensor(
        "y", [MEGA * HALVES, P, BLK], mybir.dt.float32, kind="ExternalOutput"
    )

    with TileContext(nc) as tc:
        with (
            tc.tile_pool(name="xin", bufs=3) as xpool,
            tc.tile_pool(name="outp", bufs=3) as opool,
            tc.tile_pool(name="work", bufs=3) as wpool,
        ):
            sh = [P, 3, 4, F]
            for t in range(MEGA):
                prev = None
                for h in range(HALVES):
                    xt = xpool.tile([P, BLK], mybir.dt.float32, tag="x")
                    nc.sync.dma_start(out=xt[:], in_=x.ap()[t * HALVES + h])
                    ot = opool.tile([P, BLK], mybir.dt.float32, tag="o")
                    xq = xt.rearrange("p (n k j f) -> p n k j f", n=NSUB, k=3, j=4)
                    oq = ot.rearrange("p (n i j f) -> p n i j f", n=NSUB, i=3, j=4)
                    for nl in range(NSUB):
                        if h == 0 and nl == 0:
                            nc.scalar.copy(out=oq[:, 0], in_=xq[:, 0])
                            continue
                        A = oq[:, nl - 1] if nl > 0 else prev[:, NSUB - 1]
                        C = oq[:, nl]
                        eng = nc.vector
                        tmp = wpool.tile([P, 12 * F], mybir.dt.float32, tag="tk")
                        tw = tmp.rearrange("p (i j f) -> p i j f", i=3, j=4)
                        eng.tensor_mul(
                            out=C,
                            in0=A[:, :, 0:1, :].broadcast_to(sh),
                            in1=xq[:, nl, 0:1, :, :].broadcast_to(sh),
                        )
                        eng.tensor_mul(
                            out=tw,
                            in0=A[:, :, 1:2, :].broadcast_to(sh),
                            in1=xq[:, nl, 1:2, :, :].broadcast_to(sh),
                        )
                        eng.tensor_add(out=C, in0=C, in1=tw)
                        eng.tensor_mul(
                            out=tw,
                            in0=A[:, :, 2:3, :].broadcast_to(sh),
                            in1=xq[:, nl, 2:3, :, :].broadcast_to(sh),
                        )
                        eng.tensor_add(out=C, in0=C, in1=tw)
                        eng.tensor_add(
                            out=C[:, :, 3, :], in0=C[:, :, 3, :], in1=A[:, :, 3, :]
                        )
                    nc.sync.dma_start(out=y.ap()[t * HALVES + h], in_=ot[:])
                    prev = oq
    nc.compile()
    return nc


def build(cfg: Cfg):
    if cfg.variant == "soav":
        return build_soav(cfg)
    if cfg.variant == "soa":
        return build_soa(cfg)
    if cfg.variant == "split2":
        return build_split2(cfg)
    F, NSUB, MEGA, HALVES = cfg.F, cfg.NSUB, cfg.MEGA, cfg.HALVES
    BLK = F * NSUB * 12
    nc = bacc.Bacc("TRN2", target_bir_lowering=False, debug=False)
    x = nc.dram_tensor(
        "x", [MEGA * HALVES, P, BLK], mybir.dt.float32, kind="ExternalInput"
    )
    y = nc.dram_tensor(
        "y", [MEGA * HALVES, P, BLK], mybir.dt.float32, kind="ExternalOutput"
    )

    if cfg.variant == "cumsum":
        global CUMSUM_MUL
        CUMSUM_MUL = _register_cumsum_mul()

    with TileContext(nc) as tc:
        with (
            tc.tile_pool(name="xin", bufs=3) as xpool,
            tc.tile_pool(name="outp", bufs=3) as opool,
            tc.tile_pool(name="work", bufs=3) as wpool,
            tc.tile_pool(name="scanbuf", bufs=1) as spool,
        ):
            sbufs = []
            if cfg.variant == "cumsum":
                for t in range(MEGA):
                    st = spool.tile([P, 36 * F + 3], mybir.dt.float32, tag=f"s{t}")
                    nc.vector.memset(st[:, 0:1], 0.0)
                    sbufs.append(st)

            for t in range(MEGA):
                prev = None
                for h in range(HALVES):
                    xt = xpool.tile([P, BLK], mybir.dt.float32, tag="x")
                    nc.sync.dma_start(out=xt[:], in_=x.ap()[t * HALVES + h])
                    ot = opool.tile([P, BLK], mybir.dt.float32, tag="o")
                    xv = xt.rearrange("p (f n i j) -> p f n i j", f=F, n=NSUB, i=3)
                    ov = ot.rearrange("p (f n i j) -> p f n i j", f=F, n=NSUB, i=3)
                    for nl in range(NSUB):
                        if h == 0 and nl == 0:
                            nc.scalar.copy(out=ov[:, :, 0], in_=xv[:, :, 0])
                            continue
                        A = ov[:, :, nl - 1] if nl > 0 else prev[:, :, NSUB - 1]
                        Bm = xv[:, :, nl]
                        sh = [P, F, 3, 4]
                        if cfg.variant == "dve":
                            _step_dve(nc, wpool, ov[:, :, nl], A, Bm, sh)
                        elif cfg.variant == "colsplit":
                            _step_colsplit(nc, wpool, ov[:, :, nl], A, Bm, sh)
                        else:
                            _step_cumsum(
                                nc, wpool, sbufs[t], ov[:, :, nl], A, Bm, sh
                            )
                    nc.sync.dma_start(out=y.ap()[t * HALVES + h], in_=ot[:])
                    prev = ov
    nc.compile()
    return nc


_NC_CACHE = []


def _get_nc():
    if not _NC_CACHE:
        _NC_CACHE.append(build(Cfg()))
    return _NC_CACHE[0]


def shard_input(x_full, cfg, n_cores=N_CORES):
    F, NSUB, MEGA, HALVES = cfg.F, cfg.NSUB, cfg.MEGA, cfg.HALVES
    out = []
    for c in range(n_cores):
        xc = x_full[c * cfg.B_CORE : (c + 1) * cfg.B_CORE].reshape(MEGA, P, F, N, 12)
        xc = xc.reshape(MEGA, P, F, HALVES, NSUB, 12)
        if cfg.variant in ("soa", "soav"):
            # [M, H, P, NSUB, 12, F]: each component plane contiguous in f
            xc = np.ascontiguousarray(xc.transpose(0, 3, 1, 4, 5, 2))
        else:
            xc = np.ascontiguousarray(xc.transpose(0, 3, 1, 2, 4, 5))
        out.append(xc.reshape(MEGA * HALVES, P, F * NSUB * 12))
    return out


def unshard_output(ys, cfg):
    parts = []
    for yc in ys:
        a = yc.reshape(cfg.MEGA, cfg.HALVES, P, cfg.F, cfg.NSUB, 12)
        a = a.transpose(0, 2, 3, 1, 4, 5).reshape(cfg.B_CORE, N, 3, 4)
        parts.append(a)
    return np.concatenate(parts, axis=0)


def unshard_output_split2(rts, tts, cfg):
    parts = []
    for rc, tc_ in zip(rts, tts):
        r = rc.reshape(cfg.MEGA, cfg.HALVES, P, cfg.F, cfg.NSUB, 3, 3)
        r = r.transpose(0, 2, 3, 1, 4, 5, 6).reshape(cfg.B_CORE, N, 3, 3)
        tr = tc_.reshape(cfg.MEGA, cfg.HALVES, P, cfg.F, cfg.NSUB, 3, 1)
        tr = tr.transpose(0, 2, 3, 1, 4, 5, 6).reshape(cfg.B_CORE, N, 3, 1)
        parts.append(np.concatenate([r, tr], axis=-1))
    return np.concatenate(parts, axis=0)


def unshard_output_soa(rts, tts, cfg):
    parts = []
    for rc, tc_ in zip(rts, tts):
        r = rc.reshape(cfg.MEGA, cfg.HALVES, P, cfg.NSUB, 3, 3, cfg.F)
        r = r.transpose(0, 2, 6, 1, 3, 4, 5).reshape(cfg.B_CORE, N, 3, 3)
        tr = tc_.reshape(cfg.MEGA, cfg.HALVES, P, cfg.NSUB, 3, 1, cfg.F)
        tr = tr.transpose(0, 2, 6, 1, 3, 4, 5).reshape(cfg.B_CORE, N, 3, 1)
        parts.append(np.concatenate([r, tr], axis=-1))
    return np.concatenate(parts, axis=0)


def run(x, trace=False, trace_kwargs=None):
    """Returns (out [B,N,3,4], BassKernelResults)."""
    cfg = Cfg()
    x = np.asarray(x, dtype=np.float32).reshape(B, N, 12)
    nc = _get_nc()
    in_maps = [{"x": xc} for xc in shard_input(x, cfg)]
    res = bass_utils.run_bass_kernel_spmd(
        nc,
        in_maps,
        list(range(N_CORES)),
        trace=trace,
        **(trace_kwargs or {}),
    )
    if cfg.variant == "soav":
        parts = []
        for r in res.results:
            a = r["y"].reshape(cfg.MEGA, cfg.HALVES, P, cfg.NSUB, 12, cfg.F)
            a = a.transpose(0, 2, 5, 1, 3, 4).reshape(cfg.B_CORE, N, 3, 4)
            parts.append(a)
        out = np.concatenate(parts, axis=0)
    elif cfg.variant == "soa":
        out = unshard_output_soa(
            [r["yr"] for r in res.results], [r["yt"] for r in res.results], cfg
        )
    elif cfg.variant == "split2":
        out = unshard_output_split2(
            [r["yr"] for r in res.results], [r["yt"] for r in res.results], cfg
        )
    else:
        out = unshard_output([r["y"] for r in res.results], cfg)
    return out.reshape(B, N, 3, 4), res


def kernel(x):
    return run(x)[0]



# revision 23
# speedup vs baseline: 1.1867x; 1.1867x over previous
"""SE(3) compose-scan Trainium2 kernel (nn_ComposeRt).

x [131072, 32, 3, 4] fp32 -> cumulative compose along axis 1:
out[b,0] = x[b,0]; out[b,n] = out[b,n-1] o x[b,n],
[rA|tA] o [rB|tB] = [rA@rB | tA + rA@tB].

Sharding: pure data parallel over batch across 8 NeuronCores.
Per core: batch b_local = t*(P*F) + p*F + f (mega-tile t, partition p,
slot f). DRAM I/O blocks [MEGA*HALVES, P, F*NSUB*12]; block (t, h) holds
n-range [h*NSUB, (h+1)*NSUB), SBUF layout [p][f][n][i*4+j].

Variants:
- "dve" (DEFAULT, fastest measured ~320us): per scan step, six
  vector-engine tensor ops (3 broadcast multiplies, 2 accumulate adds,
  translation add) batched over (f, i, j).
- "cumsum": the scalar engine materializes both operands of all nine
  rotation products as contiguous per-partition streams (A replicated
  over j, B replicated over i); one custom DVE op computes the running
  sum of products over the stream; a strided subtract of group
  boundaries extracts the nine dot products; a small add applies the
  carried translation. 51 instead of 63 DVE element-cycles per compose
  and 3 instead of 6 DVE instructions per step. Loses: scalar-engine
  materialization lands on the serial critical path.

Optimization post-mortem (2026-08-08 session, all measured on HW):
- The scan is fully serial in n; per-step critical path = all compose
  work. Vector-only floor ~= 31 steps * (5 ops * 1755ns + 840ns) ~= 300us;
  the "dve" variant sits within ~7% of it. DMA is ~158us (not the bound).
- "colsplit"/"split2" (gpsimd computes translation column concurrently):
  426us/394us. gpsimd TENSOR_TENSOR is ~2us per 384-elem op AND its Q7
  SBUF traffic degrades concurrent DVE ops from 1755ns to ~2100ns per
  1536 elems. gpsimd is a net loss for fine-grained elementwise work.
- "soa"/"soav" (component planes contiguous in f): DVE big ops got
  SLOWER (2100ns vs 1755ns per 1536 elems): AoS's innermost 0-stride
  broadcast dim compresses SBUF reads 4:1, beating full-rate contiguous
  reads. The AoS baseline layout is already optimal for these ops.
- Scalar (Activation) engine cannot do elementwise tensor*tensor (scale/
  bias are per-partition scalars); PE cannot batch per-slot 3x3 weights;
  custom DVE ops are capped at 2 free dims, which blocks every
  materialization-free segmented-dot stream; fp16 output overflows
  (|out|max ~4e9); bf16 compute breaks the 2e-2 gate over 31 steps.
"""

import sys

if "/opt/trn_rl_repo" not in sys.path:
    sys.path.insert(0, "/opt/trn_rl_repo")

import numpy as np

import concourse.bacc as bacc
import concourse.mybir as mybir
from concourse import bass_utils, dve_ops
from concourse.dve_ops import DveOp
from concourse.dve_spec import AluOp, Spec, Src0, Src1, lower, scan
from concourse.dve_uop import DveOpSpec
from concourse.tile import TileContext

P = 128
N = 32
N_CORES = 8
B = 131072

# tunables
VARIANT = "dve"  # "dve" | "cumsum" | "colsplit" | "split2" | "soa" | "soav"
F = 128  # batch slots per partition per mega-tile
NSUB = 2  # n per sub-tile (DMA block)
MEGA = 1  # mega-tiles per core; MEGA*P*F == B // N_CORES
HALVES = N // NSUB
B_CORE = B // N_CORES
assert MEGA * P * F == B_CORE


def _register_cumsum_mul():
    """Runtime-register the custom DVE op out[k] = sum_{u<=k} in0[u]*in1[u]."""
    if any(op.name == "CUMSUM_MUL" for op in dve_ops.OPS):
        return next(op for op in dve_ops.OPS if op.name == "CUMSUM_MUL")

    def _ref(in0, in1, s0, s1, imm2):
        prod = in0.astype(np.float32) * in1.astype(np.float32)
        flat = prod.reshape(prod.shape[0], -1)
        return np.cumsum(flat, axis=-1).reshape(prod.shape)

    spec = Spec(body=scan(AluOp.ADD, Src0 * Src1), reference=_ref)
    shas = {}
    for ver in ("v3", "v4"):
        tmp = DveOpSpec(name="CUMSUM_MUL", opcode=0, uops=lower(spec, ver=ver), rd1_en=True)
        shas[ver] = tmp.sha(ver)
    op = DveOp("CUMSUM_MUL", spec, subdim=False, uops_sha=shas)
    dve_ops.OPS.append(op)
    dve_ops.CUSTOM_DVE_SPECS[op.name] = op.spec
    dve_ops._SUB_OPCODE_FOR_NAME[op.name] = (
        dve_ops._CUSTOM_DVE_ROW_BASE + len(dve_ops.OPS) - 1
    )
    return op


CUMSUM_MUL = None  # registered lazily by build() for the cumsum variant


class Cfg:
    def __init__(self, F=F, NSUB=NSUB, MEGA=MEGA, variant=VARIANT):
        self.F = F
        self.NSUB = NSUB
        self.MEGA = MEGA
        self.HALVES = N // NSUB
        self.B_CORE = MEGA * P * F
        self.variant = variant


def _step_dve(nc, ppool, C, A, Bm, sh):
    eng = nc.vector
    F_ = sh[1]
    tmp = ppool.tile([P, F_ * 12], mybir.dt.float32, tag="tk")
    tv = tmp.rearrange("p (f i j) -> p f i j", f=F_, i=3)
    eng.tensor_mul(
        out=C,
        in0=A[:, :, :, 0:1].broadcast_to(sh),
        in1=Bm[:, :, 0:1, :].broadcast_to(sh),
    )
    eng.tensor_mul(
        out=tv,
        in0=A[:, :, :, 1:2].broadcast_to(sh),
        in1=Bm[:, :, 1:2, :].broadcast_to(sh),
    )
    eng.tensor_add(out=C, in0=C, in1=tv)
    eng.tensor_mul(
        out=tv,
        in0=A[:, :, :, 2:3].broadcast_to(sh),
        in1=Bm[:, :, 2:3, :].broadcast_to(sh),
    )
    eng.tensor_add(out=C, in0=C, in1=tv)
    eng.tensor_add(out=C[:, :, :, 3], in0=C[:, :, :, 3], in1=A[:, :, :, 3])


def _step_colsplit(nc, ppool, C, A, Bm, sh):
    """Split the compose across engines: vector computes C's columns 0-2
    (rotation @ rotation), gpsimd computes column 3 (rotation @ tB + tA).
    Vector's step-n ops read only A's columns 0-2 (its own prior output),
    so the serial scan advances at vector speed; gpsimd trails one
    semaphore behind and never blocks it."""
    F_ = sh[1]
    sh3 = [P, F_, 3, 3]
    eng = nc.vector
    C3 = C[:, :, :, 0:3]
    tmp = ppool.tile([P, F_ * 9], mybir.dt.float32, tag="tk")
    tv = tmp.rearrange("p (f i j) -> p f i j", f=F_, i=3)
    eng.tensor_mul(
        out=C3,
        in0=A[:, :, :, 0:1].broadcast_to(sh3),
        in1=Bm[:, :, 0:1, 0:3].broadcast_to(sh3),
    )
    eng.tensor_mul(
        out=tv,
        in0=A[:, :, :, 1:2].broadcast_to(sh3),
        in1=Bm[:, :, 1:2, 0:3].broadcast_to(sh3),
    )
    eng.tensor_add(out=C3, in0=C3, in1=tv)
    eng.tensor_mul(
        out=tv,
        in0=A[:, :, :, 2:3].broadcast_to(sh3),
        in1=Bm[:, :, 2:3, 0:3].broadcast_to(sh3),
    )
    eng.tensor_add(out=C3, in0=C3, in1=tv)

    g = nc.gpsimd
    shv = [P, F_, 3]
    Ct = C[:, :, :, 3]
    tg = ppool.tile([P, F_ * 3], mybir.dt.float32, tag="tg")
    tgv = tg.rearrange("p (f i) -> p f i", f=F_)
    g.tensor_mul(
        out=Ct,
        in0=A[:, :, :, 0],
        in1=Bm[:, :, 0:1, 3].broadcast_to(shv),
    )
    g.tensor_mul(
        out=tgv,
        in0=A[:, :, :, 1],
        in1=Bm[:, :, 1:2, 3].broadcast_to(shv),
    )
    g.tensor_add(out=Ct, in0=Ct, in1=tgv)
    g.tensor_mul(
        out=tgv,
        in0=A[:, :, :, 2],
        in1=Bm[:, :, 2:3, 3].broadcast_to(shv),
    )
    g.tensor_add(out=Ct, in0=Ct, in1=tgv)
    g.tensor_add(out=Ct, in0=Ct, in1=A[:, :, :, 3])


def _step_cumsum(nc, epool, sbuf_S, C, A, Bm, sh):
    """A/Bm/C: [P, F, 3, 4] views; sbuf_S: persistent [P, 36F+3] scan buffer
    with S[:,0] pre-zeroed."""
    F_ = sh[1]
    G = 36 * F_
    aexp = epool.tile([P, G], mybir.dt.float32, tag="aexp")
    bexp = epool.tile([P, G], mybir.dt.float32, tag="bexp")
    # stream position = f*36 + i*12 + j*3 + k
    for k in range(3):
        a_out = aexp.rearrange("p (f i j k2) -> p f i j k2", f=F_, i=3, j=4)[
            :, :, :, :, k
        ]
        b_out = bexp.rearrange("p (f i j k2) -> p f i j k2", f=F_, i=3, j=4)[
            :, :, :, :, k
        ]
        nc.scalar.copy(out=a_out, in_=A[:, :, :, k : k + 1].broadcast_to(sh))
        nc.scalar.copy(out=b_out, in_=Bm[:, :, k : k + 1, :].broadcast_to(sh))
    s_out = sbuf_S[:, 1 : 1 + G]
    nc.vector._custom_dve(CUMSUM_MUL, out=s_out, in0=aexp[:], in1=bexp[:])
    minu = sbuf_S[:, 3 : 3 + G].rearrange("p (f g k) -> p f g k", f=F_, g=12)[
        :, :, :, 0
    ]
    subt = sbuf_S[:, 0:G].rearrange("p (f g k) -> p f g k", f=F_, g=12)[:, :, :, 0]
    cflat = C.rearrange("p f i j -> p f (i j)")
    nc.vector.tensor_tensor(
        out=cflat, in0=minu, in1=subt, op=mybir.AluOpType.subtract
    )
    nc.vector.tensor_add(out=C[:, :, :, 3], in0=C[:, :, :, 3], in1=A[:, :, :, 3])


def build_split2(cfg: Cfg):
    """Rotation (9 comps) and translation (3 comps) in separate tiles and
    separate DRAM outputs, so vector's rotation chain and gpsimd's
    translation chain never share a written tile: vector free-runs; gpsimd
    reads vector's rotations one step behind (one-way semaphore)."""
    F, NSUB, MEGA, HALVES = cfg.F, cfg.NSUB, cfg.MEGA, cfg.HALVES
    BLKX = F * NSUB * 12
    BLKR = F * NSUB * 9
    BLKT = F * NSUB * 3
    nc = bacc.Bacc("TRN2", target_bir_lowering=False, debug=False)
    x = nc.dram_tensor(
        "x", [MEGA * HALVES, P, BLKX], mybir.dt.float32, kind="ExternalInput"
    )
    yr = nc.dram_tensor(
        "yr", [MEGA * HALVES, P, BLKR], mybir.dt.float32, kind="ExternalOutput"
    )
    yt = nc.dram_tensor(
        "yt", [MEGA * HALVES, P, BLKT], mybir.dt.float32, kind="ExternalOutput"
    )

    with TileContext(nc) as tc:
        with (
            tc.tile_pool(name="xin", bufs=3) as xpool,
            tc.tile_pool(name="rotp", bufs=3) as rpool,
            tc.tile_pool(name="trap", bufs=3) as tpool,
            tc.tile_pool(name="work", bufs=3) as wpool,
        ):
            sh3 = [P, F, 3, 3]
            shv = [P, F, 3]
            for t in range(MEGA):
                prevR = prevT = None
                for h in range(HALVES):
                    xt = xpool.tile([P, BLKX], mybir.dt.float32, tag="x")
                    nc.sync.dma_start(out=xt[:], in_=x.ap()[t * HALVES + h])
                    rt = rpool.tile([P, BLKR], mybir.dt.float32, tag="r")
                    tt = tpool.tile([P, BLKT], mybir.dt.float32, tag="t")
                    xv = xt.rearrange("p (f n i j) -> p f n i j", f=F, n=NSUB, i=3)
                    rv = rt.rearrange("p (f n i j) -> p f n i j", f=F, n=NSUB, i=3)
                    tv = tt.rearrange("p (f n i) -> p f n i", f=F, n=NSUB)
                    for nl in range(NSUB):
                        if h == 0 and nl == 0:
                            nc.scalar.copy(out=rv[:, :, 0], in_=xv[:, :, 0, :, 0:3])
                            nc.scalar.copy(out=tv[:, :, 0], in_=xv[:, :, 0, :, 3])
                            continue
                        Ar = rv[:, :, nl - 1] if nl > 0 else prevR[:, :, NSUB - 1]
                        At = tv[:, :, nl - 1] if nl > 0 else prevT[:, :, NSUB - 1]
                        Bm = xv[:, :, nl]
                        Cr = rv[:, :, nl]
                        Ct = tv[:, :, nl]
                        eng = nc.vector
                        tmp = wpool.tile([P, F * 9], mybir.dt.float32, tag="tk")
                        tw = tmp.rearrange("p (f i j) -> p f i j", f=F, i=3)
                        eng.tensor_mul(
                            out=Cr,
                            in0=Ar[:, :, :, 0:1].broadcast_to(sh3),
                            in1=Bm[:, :, 0:1, 0:3].broadcast_to(sh3),
                        )
                        eng.tensor_mul(
                            out=tw,
                            in0=Ar[:, :, :, 1:2].broadcast_to(sh3),
                            in1=Bm[:, :, 1:2, 0:3].broadcast_to(sh3),
                        )
                        eng.tensor_add(out=Cr, in0=Cr, in1=tw)
                        eng.tensor_mul(
                            out=tw,
                            in0=Ar[:, :, :, 2:3].broadcast_to(sh3),
                            in1=Bm[:, :, 2:3, 0:3].broadcast_to(sh3),
                        )
                        eng.tensor_add(out=Cr, in0=Cr, in1=tw)

                        g = nc.gpsimd
                        tg = wpool.tile([P, F * 3], mybir.dt.float32, tag="tg")
                        tgv = tg.rearrange("p (f i) -> p f i", f=F)
                        g.tensor_mul(
                            out=Ct,
                            in0=Ar[:, :, :, 0],
                            in1=Bm[:, :, 0:1, 3].broadcast_to(shv),
                        )
                        g.tensor_mul(
                            out=tgv,
                            in0=Ar[:, :, :, 1],
                            in1=Bm[:, :, 1:2, 3].broadcast_to(shv),
                        )
                        g.tensor_add(out=Ct, in0=Ct, in1=tgv)
                        g.tensor_mul(
                            out=tgv,
                            in0=Ar[:, :, :, 2],
                            in1=Bm[:, :, 2:3, 3].broadcast_to(shv),
                        )
                        g.tensor_add(out=Ct, in0=Ct, in1=tgv)
                        g.tensor_add(out=Ct, in0=Ct, in1=At)
                    nc.sync.dma_start(out=yr.ap()[t * HALVES + h], in_=rt[:])
                    nc.sync.dma_start(out=yt.ap()[t * HALVES + h], in_=tt[:])
                    prevR, prevT = rv, tv
    nc.compile()
    return nc


def build_soa(cfg: Cfg):
    """Structure-of-arrays: every transform component is a contiguous
    [P, F] plane (f innermost, 512B runs), so all vector/gpsimd ops have
    stride-1 innermost access. Rotation (9 planes/n) on vector and
    translation (3 planes/n) on gpsimd live in separate tiles/outputs;
    gpsimd trails the vector rotation chain by one step."""
    F, NSUB, MEGA, HALVES = cfg.F, cfg.NSUB, cfg.MEGA, cfg.HALVES
    BLKX = NSUB * 12 * F
    BLKR = NSUB * 9 * F
    BLKT = NSUB * 3 * F
    nc = bacc.Bacc("TRN2", target_bir_lowering=False, debug=False)
    x = nc.dram_tensor(
        "x", [MEGA * HALVES, P, BLKX], mybir.dt.float32, kind="ExternalInput"
    )
    yr = nc.dram_tensor(
        "yr", [MEGA * HALVES, P, BLKR], mybir.dt.float32, kind="ExternalOutput"
    )
    yt = nc.dram_tensor(
        "yt", [MEGA * HALVES, P, BLKT], mybir.dt.float32, kind="ExternalOutput"
    )

    with TileContext(nc) as tc:
        with (
            tc.tile_pool(name="xin", bufs=3) as xpool,
            tc.tile_pool(name="rotp", bufs=3) as rpool,
            tc.tile_pool(name="trap", bufs=3) as tpool,
            tc.tile_pool(name="work", bufs=3) as wpool,
        ):
            sh33 = [P, 3, 3, F]
            sh3 = [P, 3, F]
            for t in range(MEGA):
                prevR = prevT = None
                for h in range(HALVES):
                    xt = xpool.tile([P, BLKX], mybir.dt.float32, tag="x")
                    nc.sync.dma_start(out=xt[:], in_=x.ap()[t * HALVES + h])
                    rt = rpool.tile([P, BLKR], mybir.dt.float32, tag="r")
                    tt = tpool.tile([P, BLKT], mybir.dt.float32, tag="t")
                    xq = xt.rearrange("p (n k j f) -> p n k j f", n=NSUB, k=3, j=4)
                    rq = rt.rearrange("p (n i j f) -> p n i j f", n=NSUB, i=3, j=3)
                    tq = tt.rearrange("p (n i f) -> p n i f", n=NSUB, i=3)
                    for nl in range(NSUB):
                        if h == 0 and nl == 0:
                            nc.scalar.copy(out=rq[:, 0], in_=xq[:, 0, :, 0:3])
                            nc.scalar.copy(out=tq[:, 0], in_=xq[:, 0, :, 3])
                            continue
                        Ar = rq[:, nl - 1] if nl > 0 else prevR[:, NSUB - 1]
                        At = tq[:, nl - 1] if nl > 0 else prevT[:, NSUB - 1]
                        Cr = rq[:, nl]
                        Ct = tq[:, nl]
                        eng = nc.vector
                        tmp = wpool.tile([P, 9 * F], mybir.dt.float32, tag="tk")
                        tw = tmp.rearrange("p (i j f) -> p i j f", i=3, j=3)
                        eng.tensor_mul(
                            out=Cr,
                            in0=Ar[:, :, 0:1, :].broadcast_to(sh33),
                            in1=xq[:, nl, 0:1, 0:3, :].broadcast_to(sh33),
                        )
                        eng.tensor_mul(
                            out=tw,
                            in0=Ar[:, :, 1:2, :].broadcast_to(sh33),
                            in1=xq[:, nl, 1:2, 0:3, :].broadcast_to(sh33),
                        )
                        eng.tensor_add(out=Cr, in0=Cr, in1=tw)
                        eng.tensor_mul(
                            out=tw,
                            in0=Ar[:, :, 2:3, :].broadcast_to(sh33),
                            in1=xq[:, nl, 2:3, 0:3, :].broadcast_to(sh33),
                        )
                        eng.tensor_add(out=Cr, in0=Cr, in1=tw)

                        g = nc.gpsimd
                        tg = wpool.tile([P, 3 * F], mybir.dt.float32, tag="tg")
                        tgv = tg.rearrange("p (i f) -> p i f", i=3)
                        g.tensor_mul(
                            out=Ct,
                            in0=Ar[:, :, 0, :],
                            in1=xq[:, nl, 0:1, 3, :].broadcast_to(sh3),
                        )
                        g.tensor_mul(
                            out=tgv,
                            in0=Ar[:, :, 1, :],
                            in1=xq[:, nl, 1:2, 3, :].broadcast_to(sh3),
                        )
                        g.tensor_add(out=Ct, in0=Ct, in1=tgv)
                        g.tensor_mul(
                            out=tgv,
                            in0=Ar[:, :, 2, :],
                            in1=xq[:, nl, 2:3, 3, :].broadcast_to(sh3),
                        )
                        g.tensor_add(out=Ct, in0=Ct, in1=tgv)
                        g.tensor_add(out=Ct, in0=Ct, in1=At)
                    nc.sync.dma_start(out=yr.ap()[t * HALVES + h], in_=rt[:])
                    nc.sync.dma_start(out=yt.ap()[t * HALVES + h], in_=tt[:])
                    prevR, prevT = rq, tq
    nc.compile()
    return nc


def build_soav(cfg: Cfg):
    """All-vector SoA: 12 component planes (i-row, j-col), each [P, F]
    contiguous. Per step 5 full-width ops (3 mul + 2 add over [P,3,4,F])
    compute A_rot @ [B_rot | tB]; one contiguous-plane add applies +tA.
    gpsimd is deliberately unused: its Q7 SBUF traffic degrades DVE
    throughput by ~50% when active."""
    F, NSUB, MEGA, HALVES = cfg.F, cfg.NSUB, cfg.MEGA, cfg.HALVES
    BLK = NSUB * 12 * F
    nc = bacc.Bacc("TRN2", target_bir_lowering=False, debug=False)
    x = nc.dram_tensor(
        "x", [MEGA * HALVES, P, BLK], mybir.dt.float32, kind="ExternalInput"
    )
    y = nc.dram_tensor(
        "y", [MEGA * HALVES, P, BLK], mybir.dt.float32, kind="ExternalOutput"
    )

    with TileContext(nc) as tc:
        with (
            tc.tile_pool(name="xin", bufs=3) as xpool,
            tc.tile_pool(name="outp", bufs=3) as opool,
            tc.tile_pool(name="work", bufs=3) as wpool,
        ):
            sh = [P, 3, 4, F]
            for t in range(MEGA):
                prev = None
                for h in range(HALVES):
                    xt = xpool.tile([P, BLK], mybir.dt.float32, tag="x")
                    nc.sync.dma_start(out=xt[:], in_=x.ap()[t * HALVES + h])
                    ot = opool.tile([P, BLK], mybir.dt.float32, tag="o")
                    xq = xt.rearrange("p (n k j f) -> p n k j f", n=NSUB, k=3, j=4)
                    oq = ot.rearrange("p (n i j f) -> p n i j f", n=NSUB, i=3, j=4)
                    for nl in range(NSUB):
                        if h == 0 and nl == 0:
                            nc.scalar.copy(out=oq[:, 0], in_=xq[:, 0])
                            continue
                        A = oq[:, nl - 1] if nl > 0 else prev[:, NSUB - 1]
                        C = oq[:, nl]
                        eng = nc.vector
                        tmp = wpool.tile([P, 12 * F], mybir.dt.float32, tag="tk")
                        tw = tmp.rearrange("p (i j f) -> p i j f", i=3, j=4)
                        eng.tensor_mul(
                            out=C,
                            in0=A[:, :, 0:1, :].broadcast_to(sh),
                            in1=xq[:, nl, 0:1, :, :].broadcast_to(sh),
                        )
                        eng.tensor_mul(
                            out=tw,
                            in0=A[:, :, 1:2, :].broadcast_to(sh),
                            in1=xq[:, nl, 1:2, :, :].broadcast_to(sh),
                        )
                        eng.tensor_add(out=C, in0=C, in1=tw)
                        eng.tensor_mul(
                            out=tw,
                            in0=A[:, :, 2:3, :].broadcast_to(sh),
                            in1=xq[:, nl, 2:3, :, :].broadcast_to(sh),
                        )
                        eng.tensor_add(out=C, in0=C, in1=tw)
                        eng.tensor_add(
                            out=C[:, :, 3, :], in0=C[:, :, 3, :], in1=A[:, :, 3, :]
                        )
                    nc.sync.dma_start(out=y.ap()[t * HALVES + h], in_=ot[:])
                    prev = oq
    nc.compile()
    return nc


def build(cfg: Cfg):
    if cfg.variant == "soav":
        return build_soav(cfg)
    if cfg.variant == "soa":
        return build_soa(cfg)
    if cfg.variant == "split2":
        return build_split2(cfg)
    F, NSUB, MEGA, HALVES = cfg.F, cfg.NSUB, cfg.MEGA, cfg.HALVES
    BLK = F * NSUB * 12
    nc = bacc.Bacc("TRN2", target_bir_lowering=False, debug=False)
    x = nc.dram_tensor(
        "x", [MEGA * HALVES, P, BLK], mybir.dt.float32, kind="ExternalInput"
    )
    y = nc.dram_tensor(
        "y", [MEGA * HALVES, P, BLK], mybir.dt.float32, kind="ExternalOutput"
    )

    if cfg.variant == "cumsum":
        global CUMSUM_MUL
        CUMSUM_MUL = _register_cumsum_mul()

    with TileContext(nc) as tc:
        with (
            tc.tile_pool(name="xin", bufs=3) as xpool,
            tc.tile_pool(name="outp", bufs=3) as opool,
            tc.tile_pool(name="work", bufs=3) as wpool,
            tc.tile_pool(name="scanbuf", bufs=1) as spool,
        ):
            sbufs = []
            if cfg.variant == "cumsum":
                for t in range(MEGA):
                    st = spool.tile([P, 36 * F + 3], mybir.dt.float32, tag=f"s{t}")
                    nc.vector.memset(st[:, 0:1], 0.0)
                    sbufs.append(st)

            for t in range(MEGA):
                prev = None
                for h in range(HALVES):
                    xt = xpool.tile([P, BLK], mybir.dt.float32, tag="x")
                    nc.sync.dma_start(out=xt[:], in_=x.ap()[t * HALVES + h])
                    ot = opool.tile([P, BLK], mybir.dt.float32, tag="o")
                    xv = xt.rearrange("p (f n i j) -> p f n i j", f=F, n=NSUB, i=3)
                    ov = ot.rearrange("p (f n i j) -> p f n i j", f=F, n=NSUB, i=3)
                    for nl in range(NSUB):
                        if h == 0 and nl == 0:
                            nc.scalar.copy(out=ov[:, :, 0], in_=xv[:, :, 0])
                            continue
                        A = ov[:, :, nl - 1] if nl > 0 else prev[:, :, NSUB - 1]
                        Bm = xv[:, :, nl]
                        sh = [P, F, 3, 4]
                        if cfg.variant == "dve":
                            _step_dve(nc, wpool, ov[:, :, nl], A, Bm, sh)
                        elif cfg.variant == "colsplit":
                            _step_colsplit(nc, wpool, ov[:, :, nl], A, Bm, sh)
                        else:
                            _step_cumsum(
                                nc, wpool, sbufs[t], ov[:, :, nl], A, Bm, sh
                            )
                    nc.sync.dma_start(out=y.ap()[t * HALVES + h], in_=ot[:])
                    prev = ov
    nc.compile()
    return nc


_NC_CACHE = []


def _get_nc():
    if not _NC_CACHE:
        _NC_CACHE.append(build(Cfg()))
    return _NC_CACHE[0]


def shard_input(x_full, cfg, n_cores=N_CORES):
    F, NSUB, MEGA, HALVES = cfg.F, cfg.NSUB, cfg.MEGA, cfg.HALVES
    out = []
    for c in range(n_cores):
        xc = x_full[c * cfg.B_CORE : (c + 1) * cfg.B_CORE].reshape(MEGA, P, F, N, 12)
        xc = xc.reshape(MEGA, P, F, HALVES, NSUB, 12)
        if cfg.variant in ("soa", "soav"):
            # [M, H, P, NSUB, 12, F]: each component plane contiguous in f
            xc = np.ascontiguousarray(xc.transpose(0, 3, 1, 4, 5, 2))
        else:
            xc = np.ascontiguousarray(xc.transpose(0, 3, 1, 2, 4, 5))
        out.append(xc.reshape(MEGA * HALVES, P, F * NSUB * 12))
    return out


def unshard_output(ys, cfg):
    parts = []
    for yc in ys:
        a = yc.reshape(cfg.MEGA, cfg.HALVES, P, cfg.F, cfg.NSUB, 12)
        a = a.transpose(0, 2, 3, 1, 4, 5).reshape(cfg.B_CORE, N, 3, 4)
        parts.append(a)
    return np.concatenate(parts, axis=0)


def unshard_output_split2(rts, tts, cfg):
    parts = []
    for rc, tc_ in zip(rts, tts):
        r = rc.reshape(cfg.MEGA, cfg.HALVES, P, cfg.F, cfg.NSUB, 3, 3)
        r = r.transpose(0, 2, 3, 1, 4, 5, 6).reshape(cfg.B_CORE, N, 3, 3)
        tr = tc_.reshape(cfg.MEGA, cfg.HALVES, P, cfg.F, cfg.NSUB, 3, 1)
        tr = tr.transpose(0, 2, 3, 1, 4, 5, 6).reshape(cfg.B_CORE, N, 3, 1)
        parts.append(np.concatenate([r, tr], axis=-1))
    return np.concatenate(parts, axis=0)


def unshard_output_soa(rts, tts, cfg):
    parts = []
    for rc, tc_ in zip(rts, tts):
        r = rc.reshape(cfg.MEGA, cfg.HALVES, P, cfg.NSUB, 3, 3, cfg.F)
        r = r.transpose(0, 2, 6, 1, 3, 4, 5).reshape(cfg.B_CORE, N, 3, 3)
        tr = tc_.reshape(cfg.MEGA, cfg.HALVES, P, cfg.NSUB, 3, 1, cfg.F)
        tr = tr.transpose(0, 2, 6, 1, 3, 4, 5).reshape(cfg.B_CORE, N, 3, 1)
        parts.append(np.concatenate([r, tr], axis=-1))
    return np.concatenate(parts, axis=0)


def run(x, trace=False, trace_kwargs=None):
    """Returns (out [B,N,3,4], BassKernelResults)."""
    cfg = Cfg()
    x = np.asarray(x, dtype=np.float32).reshape(B, N, 12)
    nc = _get_nc()
    in_maps = [{"x": xc} for xc in shard_input(x, cfg)]
    res = bass_utils.run_bass_kernel_spmd(
        nc,
        in_maps,
        list(range(N_CORES)),
        trace=trace,
        **(trace_kwargs or {}),
    )
    if cfg.variant == "soav":
        parts = []
        for r in res.results:
            a = r["y"].reshape(cfg.MEGA, cfg.HALVES, P, cfg.NSUB, 12, cfg.F)
            a = a.transpose(0, 2, 5, 1, 3, 4).reshape(cfg.B_CORE, N, 3, 4)
            parts.append(a)
        out = np.concatenate(parts, axis=0)
    elif cfg.variant == "soa":
        out = unshard_output_soa(
            [r["yr"] for r in res.results], [r["yt"] for r in res.results], cfg
        )
    elif cfg.variant == "split2":
        out = unshard_output_split2(
            [r["yr"] for r in res.results], [r["yt"] for r in res.results], cfg
        )
    else:
        out = unshard_output([r["y"] for r in res.results], cfg)
    return out.reshape(B, N, 3, 4), res


def kernel(x):
    return run(x)[0]

